# revision 19
# baseline (speedup 1.0000x reference)
"""Multi-head attention (B=2, S=2048, D=1024, H=16) on 8 NeuronCores.

Sharding: core c -> batch b = c//4, head group g = c%4 (4 heads each).
Each core computes q/k/v projections for its head group (bias folded in via an
augmented ones-row on x and a bias-row on W), full softmax attention for its 4
heads, and a partial output projection out_c = attn_out_c @ Wo[rows_c].  The
host sums the 4 partials per batch and adds bo.

Kernel layout (per core, all fp32):
  - xT [1152, 2048] = [x.T; ones; zeros]   (K padded 1024+1 -> 9*128)
  - qT/kT [256, 2048] computed directly in transposed layout (heads on
    partitions, 2 tiles of 128 = 2 head-pairs), v in natural layout with an
    extra ones column per head (v_aug) so the attention-output matmul also
    accumulates the softmax denominator as row 64.
  - scoresT[sk, sq] = k @ qT per 128-row k-block, row-tiled 2 heads per pass
    (K=64 each at row groups 0/64); exp on ScalarE straight out of PSUM
    (no max subtraction: scores ~ N(0,1), max << 88).
  - outT[65, sq] accumulates over k-blocks in PSUM; row 64 is the denominator.
    Normalization: reciprocal of row 64, partition-broadcast via DMA, fused
    into the PSUM->SBUF evacuation multiply.
  - projection: out[s, n] = sum_h outTs_h[dv, s].T @ Wo_h[dv, n], K=64 chunks.
"""

import numpy as np

S = 2048
D = 1024
H = 16
DEPTH = 64
NCORES = 8
GH = 4              # heads per core
GD = GH * DEPTH     # 256 output dims per core
KC = 9              # contraction chunks of 128 (1024 data + 1 bias + pad)
KAUG = KC * 128     # 1152

_state = {}


def _build():
    import concourse.mybir as mybir
    import concourse.tile as tile
    from concourse import bacc
    from concourse.bass import ts

    fp32 = mybir.dt.float32
    # All matmul operands live as float32r (same 4-byte layout, np.float32 on
    # the host): the PE streams fp32r at 1 row/cycle vs fp32's 4, at ~tf32
    # operand precision.  PSUM accumulation stays fp32.
    fp32r = mybir.dt.float32r
    Exp = mybir.ActivationFunctionType.Exp

    nc = bacc.Bacc("TRN2", target_bir_lowering=False, debug=False)
    xT = nc.dram_tensor("xT", [KAUG, S], fp32r, kind="ExternalInput")
    wq = nc.dram_tensor("wq", [KAUG, GD], fp32r, kind="ExternalInput")
    wk = nc.dram_tensor("wk", [KAUG, GD], fp32r, kind="ExternalInput")
    wv = nc.dram_tensor("wv", [KAUG, GD], fp32r, kind="ExternalInput")
    wo = nc.dram_tensor("wo", [GD, D], fp32r, kind="ExternalInput")
    out = nc.dram_tensor("out", [S, D], fp32, kind="ExternalOutput")

    with tile.TileContext(nc) as tc:
        with tc.tile_pool(name="singles", bufs=1) as singles:
            qT = singles.tile([128, 2, S], fp32r)       # [dout%128, pair, sq]
            kT = singles.tile([128, 2, S], fp32r)
            v_sb = singles.tile([128, 16, GH, DEPTH + 1], fp32r)  # v_aug
            # unnormalized attn out^T (normalized in place later)
            outTs = singles.tile([64, GH, S], fp32r)
            wo_sb = singles.tile([64, GH, D], fp32r)
            nc.vector.memset(v_sb[:, :, :, DEPTH : DEPTH + 1].bitcast(fp32), 1.0)

            # ---------- phase 1: QKV projections ----------
            with (
                tc.tile_pool(name="wpool", bufs=1) as wpool,
                tc.tile_pool(name="xpool", bufs=2) as xpool,
                tc.tile_pool(name="ps1", bufs=3, space="PSUM") as ps1,
            ):
                wq_sb = wpool.tile([128, KC, GD], fp32r)
                wk_sb = wpool.tile([128, KC, GD], fp32r)
                wv_sb = wpool.tile([128, KC, GD], fp32r)
                xc0_head = wpool.tile([128, 512], fp32r)
                # first-needed chunks first so the first matmuls start early
                xT_view = xT[:].rearrange("(c p) s -> p c s", p=128)
                for w_sb, w in ((wq_sb, wq), (wk_sb, wk), (wv_sb, wv)):
                    nc.sync.dma_start(w_sb[:, 0, :], w[0:128, :])
                nc.sync.dma_start(xc0_head[:], xT_view[:, 0, 0:512])
                for w_sb, w in ((wq_sb, wq), (wk_sb, wk), (wv_sb, wv)):
                    nc.sync.dma_start(
                        w_sb[:, 1:KC, :],
                        w[128:KAUG, :].rearrange("(c p) d -> p c d", p=128),
                    )
                nc.sync.dma_start(
                    wo_sb[:], wo[:].rearrange("(h p) n -> p h n", p=64)
                )

                for sc in range(4):  # s-chunks of 512
                    xc = xpool.tile([128, KC, 512], fp32r, tag="xc")
                    if sc == 0:
                        nc.vector.tensor_copy(xc[:, 0, :], xc0_head[:])
                    else:
                        nc.sync.dma_start(xc[:, 0, :], xT_view[:, 0, ts(sc, 512)])
                    nc.sync.dma_start(
                        xc[:, 1:KC, :], xT_view[:, 1:KC, ts(sc, 512)]
                    )
                    for hp in range(2):
                        for w_sb, dst in ((wq_sb, qT), (wk_sb, kT)):
                            ps = ps1.tile([128, 512], fp32, tag="pq")
                            for kc in range(KC):
                                nc.tensor.matmul(
                                    ps[:],
                                    w_sb[:, kc, ts(hp, 128)],
                                    xc[:, kc, :],
                                    start=(kc == 0),
                                    stop=(kc == KC - 1),
                                )
                            nc.vector.tensor_copy(dst[:, hp, ts(sc, 512)], ps[:])
                    for mm in range(4):  # s-blocks of 128 inside the chunk
                        ps = ps1.tile([128, GD], fp32, tag="pv")
                        for kc in range(KC):
                            nc.tensor.matmul(
                                ps[:],
                                xc[:, kc, ts(mm, 128)],
                                wv_sb[:, kc, :],
                                start=(kc == 0),
                                stop=(kc == KC - 1),
                            )
                        nc.vector.tensor_copy(
                            v_sb[:, sc * 4 + mm, :, 0:DEPTH],
                            ps[:].rearrange("p (h d) -> p h d", h=GH),
                        )

            # ---------- phase 2+3: attention fused with projection ----------
            # sq-chunk outer so each 512-wide chunk finishes all 4 heads,
            # normalizes, and projects while attention continues on the next
            # chunk -- the PE never idles long enough to re-throttle.
            # softmax denominators bounce through DRAM twice: once to spread
            # the [1, 2048] row over 64 partitions (so the iterative-divide
            # reciprocal uses 64 DVE lanes instead of 1), once to broadcast
            # the result across the dv partitions.  Layout: [sqc][h][512].
            raw_dram = nc.dram_tensor("denom_raw", [GH * S], fp32)
            rec_dram = nc.dram_tensor("denom_rec", [GH * S], fp32r)
            with (
                tc.tile_pool(name="expp", bufs=3) as expp,
                tc.tile_pool(name="rbp", bufs=2) as rbp,
                tc.tile_pool(name="outp", bufs=3) as outp,
                tc.tile_pool(name="pss", bufs=2, space="PSUM") as pss,
                tc.tile_pool(name="pso", bufs=4, space="PSUM") as pso,
            ):
                for sqc in range(4):
                    # reciprocal denominators for this sq-chunk (row 64
                    # mirrors the PSUM denominator row partition)
                    rt = rbp.tile([65, GH, 512], fp32, tag="rt")
                    for hp in range(2):
                        oAB = [
                            pso.tile([65, 512], fp32, tag="o", name=f"o{a}")
                            for a in range(2)
                        ]
                        for kb in range(16):
                            sps = pss.tile([128, 2, 512], fp32, tag="s")
                            # scoresT = k @ qT, two heads row-tiled (K=64)
                            for a in range(2):
                                nc.tensor.matmul(
                                    sps[:, a, :],
                                    kT[a * 64 : (a + 1) * 64, hp, ts(kb, 128)],
                                    qT[a * 64 : (a + 1) * 64, hp, ts(sqc, 512)],
                                    start=True,
                                    stop=True,
                                )
                            ex = expp.tile([128, 2, 512], fp32r, tag="e")
                            nc.scalar.activation(ex[:], sps[:], Exp, scale=0.125)
                            # outT[65, sq] += v_aug.T @ expT  (row 64 = denom)
                            for a in range(2):
                                nc.tensor.matmul(
                                    oAB[a][:],
                                    v_sb[:, kb, 2 * hp + a, :],
                                    ex[:, a, :],
                                    start=(kb == 0),
                                    stop=(kb == 15),
                                )
                        for a in range(2):
                            h = 2 * hp + a
                            nc.vector.tensor_copy(
                                rt[64:65, h, :], oAB[a][64:65, :]
                            )
                            nc.vector.tensor_copy(
                                outTs[:, h, ts(sqc, 512)], oAB[a][0:64, :]
                            )

                    # all 4 heads of this sq-chunk done
                    raw_v = raw_dram[sqc * 2048 : (sqc + 1) * 2048].rearrange(
                        "(p h s) -> p h s", p=1, h=GH
                    )
                    nc.sync.dma_start(raw_v, rt[64:65, :, :])
                    rr = rbp.tile([64, 32], fp32, tag="rr")
                    nc.sync.dma_start(
                        rr[:],
                        raw_dram[sqc * 2048 : (sqc + 1) * 2048].rearrange(
                            "(p s) -> p s", p=64
                        ),
                    )
                    nc.vector.reciprocal(rr[:], rr[:])
                    rec_v = rec_dram[sqc * 2048 : (sqc + 1) * 2048].rearrange(
                        "(p h s) -> p h s", p=1, h=GH
                    )
                    nc.sync.dma_start(
                        rec_dram[sqc * 2048 : (sqc + 1) * 2048].rearrange(
                            "(p s) -> p s", p=64
                        ),
                        rr[:].bitcast(fp32r),
                    )
                    rb = rbp.tile([64, GH, 512], fp32r, tag="rb")
                    nc.sync.dma_start(rb[:], rec_v.to_broadcast([64, GH, 512]))
                    # normalize + project per 128-row s-block
                    for mm in range(4):
                        m = sqc * 4 + mm
                        nc.vector.tensor_mul(
                            outTs[:, :, ts(m, 128)],
                            outTs[:, :, ts(m, 128)],
                            rb[:, :, ts(mm, 128)],
                        )
                        ot = outp.tile([128, D], fp32, tag="ot")
                        for nn in range(2):  # n-chunks of 512
                            ps = pso.tile([128, 512], fp32, tag="o", name="pp")
                            for h in range(GH):
                                nc.tensor.matmul(
                                    ps[:],
                                    outTs[:, h, ts(m, 128)],
                                    wo_sb[:, h, ts(nn, 512)],
                                    start=(h == 0),
                                    stop=(h == GH - 1),
                                )
                            nc.vector.tensor_copy(ot[:, ts(nn, 512)], ps[:])
                        nc.sync.dma_start(out[m * 128 : (m + 1) * 128, :], ot[:])

    nc.compile()
    return nc


def _get_nc():
    if "nc" not in _state:
        _state["nc"] = _build()
    return _state["nc"]


def _prep_core_inputs(inputs, Wq, bq, Wk, bk, Wv, bv, Wo, bo):
    """Build the 8 per-core input dicts (host-side shard + transpose + bias fold)."""
    in_maps = []
    for c in range(NCORES):
        b, g = divmod(c, 4)
        cols = slice(g * GD, (g + 1) * GD)
        xTa = np.zeros((KAUG, S), np.float32)
        xTa[:D] = inputs[b].T
        xTa[D] = 1.0
        m = {"xT": xTa}
        for name, W, bias in (("wq", Wq, bq), ("wk", Wk, bk), ("wv", Wv, bv)):
            Wa = np.zeros((KAUG, GD), np.float32)
            Wa[:D] = W[:, cols]
            Wa[D] = bias[cols]
            m[name] = Wa
        m["wo"] = np.ascontiguousarray(Wo[cols, :], dtype=np.float32)
        in_maps.append(m)
    return in_maps


def run(inputs, Wq, bq, Wk, bk, Wv, bv, Wo, bo, trace=False):
    from concourse.bass_utils import run_bass_kernel_spmd

    nc = _get_nc()
    in_maps = _prep_core_inputs(inputs, Wq, bq, Wk, bk, Wv, bv, Wo, bo)
    res = run_bass_kernel_spmd(
        nc, in_maps, core_ids=list(range(NCORES)), trace=trace
    )
    out = np.zeros((2, S, D), np.float32)
    for c in range(NCORES):
        out[c // 4] += res.results[c]["out"]
    out += np.asarray(bo, np.float32)
    return out, res


def kernel(inputs, Wq, bq, Wk, bk, Wv, bv, Wo, bo):
    out, _ = run(
        np.asarray(inputs, np.float32),
        np.asarray(Wq, np.float32), np.asarray(bq, np.float32),
        np.asarray(Wk, np.float32), np.asarray(bk, np.float32),
        np.asarray(Wv, np.float32), np.asarray(bv, np.float32),
        np.asarray(Wo, np.float32), np.asarray(bo, np.float32),
    )
    return out
